# revision 14
# baseline (speedup 1.0000x reference)
"""Trainium2 Bass kernel for the PCNN (piecewise-CNN) bag-classification model.

Pipeline (per NeuronCore, data-parallel over sentences, 256 sentences/core):
  1. indirect-DMA gather of word/positional embeddings (token-major, bf16)
  2. PE transposes -> channel-major X tiles
  3. conv1d(k=3, edge-pad) as PSUM-accumulated matmuls over (tap, channel-chunk)
  4. PCNN piecewise max-pool: rank-1 mask matmuls into PSUM + serial reduce_max
  5. ReLU (+conv-bias fold), dense projection to 53 logits per sentence
  6. bag segment-mean as matmul with a host-built normalized selection matrix
  7. AllReduce over 8 cores, softmax, output [256, 53]

Scaling fold (exact reformulation): conv_w,conv_b are pre-scaled by 0.01 and
dense_w by 100 so the +100*mask trick of the reference becomes +1.0*mask,
keeping everything well-conditioned in bf16/fp32.
"""

import os
import sys

for _p in ("/opt/trn_rl_repo",):
    if _p not in sys.path:
        sys.path.insert(0, _p)

import numpy as np
import ml_dtypes

# ---------------- problem constants (hardcoded per spec) ----------------
N = 2048          # total sentences
L = 120           # max sentence length
LP = 122          # edge-padded length
NCORES = 8
NS = N // NCORES  # 256 sentences per core
BLK = 32          # sentences per block (SBUF-resident unit)
NBLK = NS // BLK  # 8 blocks
SGS = 4           # sentences per matmul subgroup
SG_PER_BLK = BLK // SGS          # 8
SG_COLS = 512                    # padded columns per subgroup (4*122=488 real)
BLK_COLS = SG_PER_BLK * SG_COLS  # 4096
TILES_PER_BLK = BLK_COLS // 128  # 32
NF = 230
NREL = 53
NBAGS = 256
VOCAB = 100000
WD = 300
PD = 5
NPOS = 240
FCH = [(0, 128), (128, 102)]          # filter chunks
CCH = [(0, 128), (128, 128), (256, 54)]  # channel chunks over [word(300), pf1(5), pf2(5)]

BF16 = ml_dtypes.bfloat16

_PROGRAM = None  # cached (nc,) across calls
LAST_RESULT = None


def _build_program():
    import concourse.bass as bass
    import concourse.mybir as mybir
    import concourse.tile as tile
    from concourse import bacc
    from concourse.masks import make_identity

    f32 = mybir.dt.float32
    bf16 = mybir.dt.bfloat16
    i32 = mybir.dt.int32
    AF = mybir.ActivationFunctionType
    AX = mybir.AxisListType

    nc = bacc.Bacc(
        "TRN2",
        target_bir_lowering=False,
        debug=False,
        num_devices=NCORES,
    )

    # ------------- external I/O -------------
    wemb = nc.dram_tensor("wemb", [VOCAB, WD], bf16, kind="ExternalInput").ap()
    pptab = nc.dram_tensor("pptab", [NPOS * NPOS, 2 * PD], bf16, kind="ExternalInput").ap()
    idxw_d = nc.dram_tensor("idxw", [128, NBLK * TILES_PER_BLK], i32, kind="ExternalInput").ap()
    idxp_d = nc.dram_tensor("idxp", [128, NBLK * TILES_PER_BLK], i32, kind="ExternalInput").ap()
    masks_d = nc.dram_tensor("masksd", [3 * NBLK, BLK * L], bf16, kind="ExternalInput").ap()
    snorm_d = nc.dram_tensor("snorm", [NS, NBAGS], bf16, kind="ExternalInput").ap()
    wt_d = nc.dram_tensor("wt", [3, 128, 3 * NF], bf16, kind="ExternalInput").ap()
    dwt_d = nc.dram_tensor("dwt", [128, 6 * NREL], bf16, kind="ExternalInput").ap()
    actb_d = nc.dram_tensor("actb", [128, 2], f32, kind="ExternalInput").ap()
    dbias_d = nc.dram_tensor("dbias", [1, NREL], bf16, kind="ExternalInput").ap()
    out_d = nc.dram_tensor("out", [NBAGS, NREL], f32, kind="ExternalOutput").ap()
    debug = bool(int(os.environ.get("KERNEL_DEBUG", "0")))
    if debug:
        dbg_xg = nc.dram_tensor("dbg_xg", [128, TILES_PER_BLK, WD + 2 * PD], bf16,
                                kind="ExternalOutput").ap()
        dbg_xc = nc.dram_tensor("dbg_xc", [3, 128, BLK_COLS], bf16,
                                kind="ExternalOutput").ap()
        dbg_pooled = nc.dram_tensor("dbg_pooled", [2, 128, 3, NS], f32,
                                    kind="ExternalOutput").ap()
        dbg_bag = nc.dram_tensor("dbg_bag", [NBAGS, NREL], f32,
                                 kind="ExternalOutput").ap()

    with tile.TileContext(nc) as tc:
        import contextlib

        ctx = contextlib.ExitStack()
        with ctx:
            singles = ctx.enter_context(tc.tile_pool(name="singles", bufs=1))

            # persistent tiles
            wt_sb = [singles.tile([128, 3 * NF], bf16, name=f"wt{c}") for c in range(3)]
            sel = [singles.tile([3, 128], bf16, name=f"sel{j}") for j in range(3)]
            snorm_sb = [singles.tile([128, NBAGS], bf16, name=f"sn{c}") for c in range(2)]
            idxw_sb = singles.tile([128, NBLK * TILES_PER_BLK], i32)
            idxp_sb = singles.tile([128, NBLK * TILES_PER_BLK], i32)
            dwt_sb = singles.tile([128, 6 * NREL], bf16)
            actb_sb = singles.tile([128, 2], f32)
            dbias_sb = singles.tile([1, NREL], bf16)
            ident = singles.tile([128, 128], bf16)
            ones_sb = singles.tile([1, 128], bf16)
            pooled = [singles.tile([128, 3, NS], f32, name=f"pool{c}") for c in range(2)]

            for c in range(3):
                nc.sync.dma_start(out=wt_sb[c][:, :], in_=wt_d[c, :, :])
            for c in range(2):
                nc.sync.dma_start(out=snorm_sb[c][:, :], in_=snorm_d[c * 128:(c + 1) * 128, :])
            nc.sync.dma_start(out=idxw_sb[:, :], in_=idxw_d[:, :])
            nc.sync.dma_start(out=idxp_sb[:, :], in_=idxp_d[:, :])
            nc.sync.dma_start(out=dwt_sb[:, :], in_=dwt_d[:, :])
            nc.sync.dma_start(out=actb_sb[:, :], in_=actb_d[:, :])
            nc.sync.dma_start(out=dbias_sb[:, :], in_=dbias_d[:, :])
            make_identity(nc, ident[:, :])
            pidx = singles.tile([3, 128], mybir.dt.int32, name="pidx")
            nc.gpsimd.iota(pidx[:, :], pattern=[[0, 128]], base=0, channel_multiplier=1)
            for j in range(3):
                nc.vector.tensor_scalar(
                    out=sel[j][:, :], in0=pidx[:, :], scalar1=j, scalar2=None,
                    op0=mybir.AluOpType.is_equal,
                )
            nc.vector.memset(ones_sb[:, :], 1.0)
            nc.vector.memset(pooled[0][:, :, :], 0.0)
            nc.vector.memset(pooled[1][:, :, :], 0.0)

            xg_pool = ctx.enter_context(tc.tile_pool(name="xg", bufs=2))
            mask_pool = ctx.enter_context(tc.tile_pool(name="mask", bufs=2))
            xc_pool = ctx.enter_context(tc.tile_pool(name="xc", bufs=2))
            tp_psum = ctx.enter_context(tc.tile_pool(name="tp", bufs=2, space="PSUM"))
            cv_psum = ctx.enter_context(tc.tile_pool(name="cv", bufs=4, space="PSUM"))

            for blk in range(NBLK):
                # ---- gather (token-major) ----
                xg = xg_pool.tile([128, TILES_PER_BLK, WD + 2 * PD], bf16, tag="xg")
                for t in range(TILES_PER_BLK):
                    col = blk * TILES_PER_BLK + t
                    nc.gpsimd.indirect_dma_start(
                        out=xg[:, t, 0:WD],
                        out_offset=None,
                        in_=wemb[:, :],
                        in_offset=bass.IndirectOffsetOnAxis(
                            ap=idxw_sb[:, col:col + 1], axis=0),
                    )
                    nc.gpsimd.indirect_dma_start(
                        out=xg[:, t, WD:WD + 2 * PD],
                        out_offset=None,
                        in_=pptab[:, :],
                        in_offset=bass.IndirectOffsetOnAxis(
                            ap=idxp_sb[:, col:col + 1], axis=0),
                    )
                mask_sb = mask_pool.tile([3, BLK * L], bf16, tag="mask")
                nc.sync.dma_start(out=mask_sb[:, :], in_=masks_d[blk * 3:(blk + 1) * 3, :])

                if debug and blk == 0:
                    nc.sync.dma_start(out=dbg_xg[:, :, :], in_=xg[:, :, :])
                # ---- transpose to channel-major ----
                xc = [
                    xc_pool.tile([128, BLK_COLS], bf16, tag=f"xc{c}", name=f"xc{c}")
                    for c in range(3)
                ]
                for grp in range(SG_PER_BLK):  # 4 token-tiles per group
                    for cc, (c0, cw) in enumerate(CCH):
                        tp = tp_psum.tile([128, 4, 128], bf16, tag="tp", name=f"tp{cc}")
                        for t in range(4):
                            ti = grp * 4 + t
                            nc.tensor.transpose(
                                out=tp[0:cw, t, :],
                                in_=xg[:, ti, c0:c0 + cw],
                                identity=ident[:, :],
                            )
                        nc.vector.tensor_copy(
                            out=xc[cc][0:cw, grp * 512:(grp + 1) * 512],
                            in_=tp[0:cw, :, :],
                        )

                if debug and blk == 0:
                    for c in range(3):
                        nc.sync.dma_start(out=dbg_xc[c, :, :], in_=xc[c][:, :])
                # ---- conv + piecewise max-pool ----
                for sg in range(SG_PER_BLK):
                    for fc, (f0, fw) in enumerate(FCH):
                        ps = cv_psum.tile([128, SGS, L], f32, tag="cv")
                        nmm = 0
                        for k in range(3):
                            for cc, (c0, cw) in enumerate(CCH):
                                base = xc[cc][0:cw, sg * SG_COLS + k:sg * SG_COLS + k + 1]
                                rhs = bass.AP(
                                    tensor=base.tensor,
                                    offset=base.offset,
                                    ap=[base.ap[0], [LP, SGS], [1, L]],
                                )
                                nc.tensor.matmul(
                                    out=ps[0:fw, :, :],
                                    lhsT=wt_sb[cc][0:cw, k * NF + f0:k * NF + f0 + fw],
                                    rhs=rhs,
                                    start=(nmm == 0),
                                    stop=False,
                                    skip_group_check=True,
                                )
                                nmm += 1
                        for j in range(3):
                            nc.tensor.matmul(
                                out=ps[0:fw, :, :],
                                lhsT=sel[j][:, 0:fw],
                                rhs=mask_sb[:, sg * SGS * L:(sg + 1) * SGS * L],
                                start=False,
                                stop=(j == 2),
                                skip_group_check=True,
                            )
                            s0 = blk * BLK + sg * SGS
                            nc.vector.reduce_max(
                                out=pooled[fc][0:fw, j, s0:s0 + SGS],
                                in_=ps[0:fw, :, :],
                                axis=AX.X,
                            )

            # ---------------- tail ----------------
            if debug:
                for fc in range(2):
                    nc.sync.dma_start(out=dbg_pooled[fc, :, :, :], in_=pooled[fc][:, :, :])
            # ReLU(max - 1 + 0.01*conv_b), cast to bf16
            pr = [singles.tile([128, 3, NS], bf16, name=f"pr{c}") for c in range(2)]
            for fc in range(2):
                nc.scalar.activation(
                    out=pr[fc][:, :, :],
                    in_=pooled[fc][:, :, :],
                    func=AF.Relu,
                    bias=actb_sb[:, fc:fc + 1],
                    scale=1.0,
                )

            # dense: logitsT [53, 256] = sum_{j,fc} dwt[(j,fc)].T @ pooled_r
            lg_ps = cv_psum.tile([NREL, NS], f32, tag="cv", name="lgps")
            nmm = 0
            for j in range(3):
                for fc, (f0, fw) in enumerate(FCH):
                    nc.tensor.matmul(
                        out=lg_ps[:, :],
                        lhsT=dwt_sb[0:fw, (j * 2 + fc) * NREL:(j * 2 + fc + 1) * NREL],
                        rhs=pr[fc][0:fw, j, :],
                        start=(nmm == 0),
                        stop=(nmm == 5),
                    )
                    nmm += 1
            lg_sb = singles.tile([NREL, NS], bf16)
            nc.vector.tensor_copy(out=lg_sb[:, :], in_=lg_ps[:, :])

            # transpose logits -> [256 sents, 53]
            ls = [singles.tile([128, NREL], bf16, name=f"ls{c}") for c in range(2)]
            for sc in range(2):
                ltp = tp_psum.tile([128, 4, 128], bf16, tag="tp", name="ltp")
                nc.tensor.transpose(
                    out=ltp[0:128, 0, 0:NREL],
                    in_=lg_sb[:, sc * 128:(sc + 1) * 128],
                    identity=ident[0:NREL, 0:NREL],
                )
                nc.vector.tensor_copy(out=ls[sc][:, :], in_=ltp[0:128, 0, 0:NREL])

            # bag aggregation: bagT [128 bags, 53] per bag-chunk (+ dense bias/8)
            cc_dram = ctx.enter_context(tc.tile_pool(name="ccd", bufs=1, space="DRAM"))
            cc_in = cc_dram.tile([NBAGS, NREL], f32)
            cc_out = cc_dram.tile([NBAGS, NREL], f32)
            for bc in range(2):
                bg = cv_psum.tile([128, NREL], f32, tag="cv", name="bg")
                for sc in range(2):
                    nc.tensor.matmul(
                        out=bg[:, :],
                        lhsT=snorm_sb[sc][:, bc * 128:(bc + 1) * 128],
                        rhs=ls[sc][:, :],
                        start=(sc == 0),
                        stop=False,
                    )
                nc.tensor.matmul(
                    out=bg[:, :],
                    lhsT=ones_sb[0:1, 0:128],
                    rhs=dbias_sb[0:1, :],
                    start=False,
                    stop=True,
                )
                bg_sb = singles.tile([128, NREL], f32, name=f"bgs{bc}")
                nc.vector.tensor_copy(out=bg_sb[:, :], in_=bg[:, :])
                nc.sync.dma_start(out=cc_in[bc * 128:(bc + 1) * 128, :], in_=bg_sb[:, :])

            if debug:
                nc.sync.dma_start(out=dbg_bag[:, :], in_=cc_in[:, :])
            nc.gpsimd.collective_compute(
                "AllReduce",
                mybir.AluOpType.add,
                replica_groups=[list(range(NCORES))],
                ins=[cc_in.opt()],
                outs=[cc_out.opt()],
            )

            # softmax over the 53 relations
            for bc in range(2):
                t = singles.tile([128, NREL], f32, name=f"sm{bc}")
                nc.sync.dma_start(out=t[:, :], in_=cc_out[bc * 128:(bc + 1) * 128, :])
                nmax = singles.tile([128, 1], f32, name=f"nmax{bc}")
                nc.vector.reduce_max(out=nmax[:, :], in_=t[:, :], axis=AX.X, negate=True)
                ex = singles.tile([128, NREL], f32, name=f"ex{bc}")
                nc.scalar.activation(
                    out=ex[:, :], in_=t[:, :], func=AF.Exp, bias=nmax[:, :], scale=1.0
                )
                ssum = singles.tile([128, 1], f32, name=f"ssum{bc}")
                nc.vector.reduce_sum(out=ssum[:, :], in_=ex[:, :], axis=AX.X)
                rcp = singles.tile([128, 1], f32, name=f"rcp{bc}")
                nc.vector.reciprocal(out=rcp[:, :], in_=ssum[:, :])
                res = singles.tile([128, NREL], f32, name=f"res{bc}")
                nc.vector.tensor_scalar_mul(res[:, :], ex[:, :], rcp[:, :])
                nc.sync.dma_start(out=out_d[bc * 128:(bc + 1) * 128, :], in_=res[:, :])

    nc.compile()
    return nc


def _get_program():
    global _PROGRAM
    if _PROGRAM is None:
        _PROGRAM = _build_program()
    return _PROGRAM


def _pad_edge(a):
    return np.concatenate([a[:, :1], a, a[:, -1:]], axis=1)


def _token_layout(padded):
    """[NS, LP] int32 -> gather-index layout [128, NBLK*TILES_PER_BLK].

    Within each block: 8 subgroups of 4 sentences, each padded to 512 cols
    (pad index 0). idx[p, blk*32+i] = stream[blk][i*128+p]."""
    a = padded.reshape(NBLK, SG_PER_BLK, SGS * LP)
    tok = np.zeros((NBLK, SG_PER_BLK, SG_COLS), np.int32)
    tok[:, :, :SGS * LP] = a
    flat = tok.reshape(NBLK, TILES_PER_BLK, 128)
    return flat.transpose(2, 0, 1).reshape(128, NBLK * TILES_PER_BLK)


def kernel(**inputs):
    sentences = np.asarray(inputs["sentences"]).astype(np.int32)
    pos1 = np.asarray(inputs["pos1"]).astype(np.int32)
    pos2 = np.asarray(inputs["pos2"]).astype(np.int32)
    masks = np.asarray(inputs["masks"]).astype(np.float32)
    bag_ids = np.asarray(inputs["bag_ids"]).astype(np.int64)
    word_emb = np.asarray(inputs["word_emb"]).astype(np.float32)
    pf1_emb = np.asarray(inputs["pf1_emb"]).astype(np.float32)
    pf2_emb = np.asarray(inputs["pf2_emb"]).astype(np.float32)
    conv_w = np.asarray(inputs["conv_w"]).astype(np.float32)
    conv_b = np.asarray(inputs["conv_b"]).astype(np.float32)
    dense_w = np.asarray(inputs["dense_w"]).astype(np.float32)
    dense_b = np.asarray(inputs["dense_b"]).astype(np.float32)

    # ---- shared (replicated) parameter prep ----
    wemb_bf = word_emb.astype(BF16)
    pptab = np.concatenate([
        np.broadcast_to(pf1_emb[:, None, :], (NPOS, NPOS, PD)),
        np.broadcast_to(pf2_emb[None, :, :], (NPOS, NPOS, PD)),
    ], axis=2).reshape(NPOS * NPOS, 2 * PD).astype(BF16)

    w01 = (conv_w * 0.01).transpose(1, 0, 2)  # [310, 230, 3]
    wt = np.zeros((3, 128, 3 * NF), np.float32)
    for cc, (c0, cw) in enumerate(CCH):
        wt[cc, :cw, :] = w01[c0:c0 + cw].transpose(0, 2, 1).reshape(cw, 3 * NF)
    wt = wt.astype(BF16)

    dw100 = dense_w * 100.0  # [53, 690]
    dwt = np.zeros((128, 6 * NREL), np.float32)
    for j in range(3):
        for fc, (f0, fw) in enumerate(FCH):
            dwt[:fw, (j * 2 + fc) * NREL:(j * 2 + fc + 1) * NREL] = \
                dw100[:, j * NF + f0:j * NF + f0 + fw].T
    dwt = dwt.astype(BF16)

    actb = np.full((128, 2), -1.0, np.float32)
    for fc, (f0, fw) in enumerate(FCH):
        actb[:fw, fc] = 0.01 * conv_b[f0:f0 + fw] - 1.0

    dbias = (dense_b / NCORES).reshape(1, NREL).astype(BF16)

    counts = np.bincount(bag_ids, minlength=NBAGS).astype(np.float32)
    counts = np.maximum(counts, 1.0)

    # ---- per-core prep ----
    in_maps = []
    for r in range(NCORES):
        sl = slice(r * NS, (r + 1) * NS)
        idxw = _token_layout(_pad_edge(sentences[sl]))
        t1 = _token_layout(_pad_edge(pos1[sl]))          # [128, 256]
        t2 = _token_layout(_pad_edge(pos2[sl]))
        idxp = (t1 * NPOS + t2).astype(np.int32)

        m = masks[sl]  # [256, 3, 120]
        md = np.stack([m[:, 0], m[:, 1] - m[:, 0], m[:, 2] - m[:, 1]], axis=1)
        masksd = md.reshape(NBLK, BLK, 3, L).transpose(0, 2, 1, 3) \
                   .reshape(3 * NBLK, BLK * L).astype(BF16)

        bags = bag_ids[sl]
        snorm = np.zeros((NS, NBAGS), np.float32)
        snorm[np.arange(NS), bags] = 1.0 / counts[bags]
        snorm = snorm.astype(BF16)

        in_maps.append({
            "wemb": wemb_bf,
            "pptab": pptab,
            "idxw": idxw.astype(np.int32),
            "idxp": idxp,
            "masksd": masksd,
            "snorm": snorm,
            "wt": wt,
            "dwt": dwt,
            "actb": actb,
            "dbias": dbias,
        })

    nc = _get_program()
    from concourse.bass_utils import run_bass_kernel_spmd

    trace = bool(int(os.environ.get("KERNEL_TRACE", "0")))
    res = run_bass_kernel_spmd(
        nc, in_maps, core_ids=list(range(NCORES)), trace=trace
    )
    global LAST_RESULT
    LAST_RESULT = res
    return res.results[0]["out"].astype(np.float32)


if __name__ == "__main__":
    d = np.load("/root/problem/ref_inputs.npz")
    out = kernel(**{k: d[k] for k in d.files})
    print("out", out.shape, out.dtype)
    ref = np.load("/root/problem/ref_out.npy")
    err = np.abs(out - ref).max() / np.abs(ref).max()
    print("Relative error:", err)


# revision 16
# speedup vs baseline: 1.0530x; 1.0530x over previous
"""Trainium2 Bass kernel for the PCNN (piecewise-CNN) bag-classification model.

Pipeline (per NeuronCore, data-parallel over sentences, 256 sentences/core):
  1. indirect-DMA gather of word/positional embeddings (token-major, bf16)
  2. PE transposes -> channel-major X tiles
  3. conv1d(k=3, edge-pad) as PSUM-accumulated matmuls over (tap, channel-chunk)
  4. PCNN piecewise max-pool: rank-1 mask matmuls into PSUM + serial reduce_max
  5. ReLU (+conv-bias fold), dense projection to 53 logits per sentence
  6. bag segment-mean as matmul with a host-built normalized selection matrix
  7. AllReduce over 8 cores, softmax, output [256, 53]

Scaling fold (exact reformulation): conv_w,conv_b are pre-scaled by 0.01 and
dense_w by 100 so the +100*mask trick of the reference becomes +1.0*mask,
keeping everything well-conditioned in bf16/fp32.
"""

import os
import sys

for _p in ("/opt/trn_rl_repo",):
    if _p not in sys.path:
        sys.path.insert(0, _p)

import numpy as np
import ml_dtypes

# ---------------- problem constants (hardcoded per spec) ----------------
N = 2048          # total sentences
L = 120           # max sentence length
LP = 122          # edge-padded length
NCORES = 8
NS = N // NCORES  # 256 sentences per core
BLK = 32          # sentences per block (SBUF-resident unit)
NBLK = NS // BLK  # 8 blocks
SGS = 4           # sentences per matmul subgroup
SG_PER_BLK = BLK // SGS          # 8
SG_COLS = 512                    # padded columns per subgroup (4*122=488 real)
BLK_COLS = SG_PER_BLK * SG_COLS  # 4096
TILES_PER_BLK = BLK_COLS // 128  # 32
NF = 230
NREL = 53
NBAGS = 256
VOCAB = 100000
WD = 300
PD = 5
NPOS = 240
FCH = [(0, 128), (128, 102)]          # filter chunks
CCH = [(0, 128), (128, 128), (256, 54)]  # channel chunks over [word(300), pf1(5), pf2(5)]

BF16 = ml_dtypes.bfloat16

_PROGRAM = None  # cached (nc,) across calls
LAST_RESULT = None


def _build_program():
    import concourse.bass as bass
    import concourse.mybir as mybir
    import concourse.tile as tile
    from concourse import bacc
    from concourse.masks import make_identity

    f32 = mybir.dt.float32
    bf16 = mybir.dt.bfloat16
    i32 = mybir.dt.int32
    AF = mybir.ActivationFunctionType
    AX = mybir.AxisListType

    nc = bacc.Bacc(
        "TRN2",
        target_bir_lowering=False,
        debug=False,
        num_devices=NCORES,
    )

    # ------------- external I/O -------------
    wemb = nc.dram_tensor("wemb", [VOCAB, WD], bf16, kind="ExternalInput").ap()
    xpf_d = nc.dram_tensor("xpf", [NBLK, 2 * PD, BLK_COLS], bf16, kind="ExternalInput").ap()
    idxw_d = nc.dram_tensor("idxw", [128, NBLK * TILES_PER_BLK], i32, kind="ExternalInput").ap()
    masks_d = nc.dram_tensor("masksd", [3 * NBLK, BLK * L], bf16, kind="ExternalInput").ap()
    snorm_d = nc.dram_tensor("snorm", [NS, NBAGS], bf16, kind="ExternalInput").ap()
    wt_d = nc.dram_tensor("wt", [3, 128, 3 * NF], bf16, kind="ExternalInput").ap()
    dwt_d = nc.dram_tensor("dwt", [128, 6 * NREL], bf16, kind="ExternalInput").ap()
    actb_d = nc.dram_tensor("actb", [128, 2], f32, kind="ExternalInput").ap()
    dbias_d = nc.dram_tensor("dbias", [1, NREL], bf16, kind="ExternalInput").ap()
    out_d = nc.dram_tensor("out", [NBAGS, NREL], f32, kind="ExternalOutput").ap()
    debug = bool(int(os.environ.get("KERNEL_DEBUG", "0")))
    if debug:
        dbg_xg = nc.dram_tensor("dbg_xg", [128, TILES_PER_BLK, WD + 2 * PD], bf16,
                                kind="ExternalOutput").ap()
        dbg_xc = nc.dram_tensor("dbg_xc", [3, 128, BLK_COLS], bf16,
                                kind="ExternalOutput").ap()
        dbg_pooled = nc.dram_tensor("dbg_pooled", [2, 128, 3, NS], f32,
                                    kind="ExternalOutput").ap()
        dbg_bag = nc.dram_tensor("dbg_bag", [NBAGS, NREL], f32,
                                 kind="ExternalOutput").ap()

    with tile.TileContext(nc) as tc:
        import contextlib

        ctx = contextlib.ExitStack()
        with ctx:
            singles = ctx.enter_context(tc.tile_pool(name="singles", bufs=1))

            # persistent tiles
            wt_sb = [singles.tile([128, 3 * NF], bf16, name=f"wt{c}") for c in range(3)]
            sel = [singles.tile([3, 128], bf16, name=f"sel{j}") for j in range(3)]
            snorm_sb = [singles.tile([128, NBAGS], bf16, name=f"sn{c}") for c in range(2)]
            idxw_sb = singles.tile([128, NBLK * TILES_PER_BLK], i32)
            dwt_sb = singles.tile([128, 6 * NREL], bf16)
            actb_sb = singles.tile([128, 2], f32)
            dbias_sb = singles.tile([1, NREL], bf16)
            ident = singles.tile([128, 128], bf16)
            ones_sb = singles.tile([1, 128], bf16)
            pooled = [singles.tile([128, 3, NS], f32, name=f"pool{c}") for c in range(2)]

            for c in range(3):
                nc.sync.dma_start(out=wt_sb[c][:, :], in_=wt_d[c, :, :])
            for c in range(2):
                nc.sync.dma_start(out=snorm_sb[c][:, :], in_=snorm_d[c * 128:(c + 1) * 128, :])
            nc.sync.dma_start(out=idxw_sb[:, :], in_=idxw_d[:, :])
            nc.sync.dma_start(out=dwt_sb[:, :], in_=dwt_d[:, :])
            nc.sync.dma_start(out=actb_sb[:, :], in_=actb_d[:, :])
            nc.sync.dma_start(out=dbias_sb[:, :], in_=dbias_d[:, :])
            make_identity(nc, ident[:, :])
            pidx = singles.tile([3, 128], mybir.dt.int32, name="pidx")
            nc.gpsimd.iota(pidx[:, :], pattern=[[0, 128]], base=0, channel_multiplier=1)
            for j in range(3):
                nc.vector.tensor_scalar(
                    out=sel[j][:, :], in0=pidx[:, :], scalar1=j, scalar2=None,
                    op0=mybir.AluOpType.is_equal,
                )
            nc.vector.memset(ones_sb[:, :], 1.0)
            nc.vector.memset(pooled[0][:, :, :], 0.0)
            nc.vector.memset(pooled[1][:, :, :], 0.0)

            xg_pool = ctx.enter_context(tc.tile_pool(name="xg", bufs=2))
            mask_pool = ctx.enter_context(tc.tile_pool(name="mask", bufs=2))
            xc_pool = ctx.enter_context(tc.tile_pool(name="xc", bufs=2))
            tp_psum = ctx.enter_context(tc.tile_pool(name="tp", bufs=2, space="PSUM"))
            cv_psum = ctx.enter_context(tc.tile_pool(name="cv", bufs=6, space="PSUM"))

            for blk in range(NBLK):
                # ---- gather (token-major) ----
                xg = xg_pool.tile([128, TILES_PER_BLK, WD], bf16, tag="xg")
                for t in range(TILES_PER_BLK):
                    col = blk * TILES_PER_BLK + t
                    nc.gpsimd.indirect_dma_start(
                        out=xg[:, t, 0:WD],
                        out_offset=None,
                        in_=wemb[:, :],
                        in_offset=bass.IndirectOffsetOnAxis(
                            ap=idxw_sb[:, col:col + 1], axis=0),
                    )
                mask_sb = mask_pool.tile([3, BLK * L], bf16, tag="mask")
                nc.sync.dma_start(out=mask_sb[:, :], in_=masks_d[blk * 3:(blk + 1) * 3, :])

                if debug and blk == 0:
                    nc.sync.dma_start(out=dbg_xg[:, :, :], in_=xg[:, :, :])
                # ---- transpose to channel-major ----
                xc = [
                    xc_pool.tile([128, BLK_COLS], bf16, tag=f"xc{c}", name=f"xc{c}")
                    for c in range(3)
                ]
                nc.sync.dma_start(out=xc[2][44:54, :], in_=xpf_d[blk, :, :])
                for grp in range(SG_PER_BLK):  # 4 token-tiles per group
                    for cc, (c0, cw) in enumerate(CCH):
                        pw = cw if cc < 2 else 44
                        tp = tp_psum.tile([128, 4, 128], bf16, tag="tp", name=f"tp{cc}")
                        for t in range(4):
                            ti = grp * 4 + t
                            nc.tensor.transpose(
                                out=tp[0:pw, t, :],
                                in_=xg[:, ti, c0:c0 + pw],
                                identity=ident[:, :],
                            )
                        if (grp + cc) % 2 == 0:
                            nc.vector.tensor_copy(
                                out=xc[cc][0:pw, grp * 512:(grp + 1) * 512],
                                in_=tp[0:pw, :, :],
                            )
                        else:
                            nc.scalar.copy(
                                out=xc[cc][0:pw, grp * 512:(grp + 1) * 512],
                                in_=tp[0:pw, :, :],
                            )

                if debug and blk == 0:
                    for c in range(3):
                        nc.sync.dma_start(out=dbg_xc[c, :, :], in_=xc[c][:, :])
                # ---- conv + piecewise max-pool (phase-batched x4 units) ----
                units = [(sg, fc) for sg in range(SG_PER_BLK) for fc in range(2)]
                for g in range(0, len(units), 4):
                    grp_units = units[g:g + 4]
                    tiles = {}
                    for (sg, fc) in grp_units:
                        f0, fw = FCH[fc]
                        ps = cv_psum.tile([128, SGS, L], f32, tag="cv",
                                          name=f"cv{sg}_{fc}")
                        tiles[(sg, fc)] = ps
                        nmm = 0
                        for k in range(3):
                            for cc, (c0, cw) in enumerate(CCH):
                                base = xc[cc][0:cw, sg * SG_COLS + k:sg * SG_COLS + k + 1]
                                rhs = bass.AP(
                                    tensor=base.tensor,
                                    offset=base.offset,
                                    ap=[base.ap[0], [LP, SGS], [1, L]],
                                )
                                nc.tensor.matmul(
                                    out=ps[0:fw, :, :],
                                    lhsT=wt_sb[cc][0:cw, k * NF + f0:k * NF + f0 + fw],
                                    rhs=rhs,
                                    start=(nmm == 0),
                                    stop=False,
                                    skip_group_check=True,
                                )
                                nmm += 1
                        nc.tensor.matmul(
                            out=ps[0:fw, :, :],
                            lhsT=sel[0][:, 0:fw],
                            rhs=mask_sb[:, sg * SGS * L:(sg + 1) * SGS * L],
                            start=False,
                            stop=False,
                            skip_group_check=True,
                        )
                    for j in range(3):
                        for (sg, fc) in grp_units:
                            f0, fw = FCH[fc]
                            ps = tiles[(sg, fc)]
                            s0 = blk * BLK + sg * SGS
                            nc.vector.reduce_max(
                                out=pooled[fc][0:fw, j, s0:s0 + SGS],
                                in_=ps[0:fw, :, :],
                                axis=AX.X,
                            )
                            if j < 2:
                                nc.tensor.matmul(
                                    out=ps[0:fw, :, :],
                                    lhsT=sel[j + 1][:, 0:fw],
                                    rhs=mask_sb[:, sg * SGS * L:(sg + 1) * SGS * L],
                                    start=False,
                                    stop=(j == 1),
                                    skip_group_check=True,
                                )

            # ---------------- tail ----------------
            if debug:
                for fc in range(2):
                    nc.sync.dma_start(out=dbg_pooled[fc, :, :, :], in_=pooled[fc][:, :, :])
            # ReLU(max - 1 + 0.01*conv_b), cast to bf16
            pr = [singles.tile([128, 3, NS], bf16, name=f"pr{c}") for c in range(2)]
            for fc in range(2):
                nc.scalar.activation(
                    out=pr[fc][:, :, :],
                    in_=pooled[fc][:, :, :],
                    func=AF.Relu,
                    bias=actb_sb[:, fc:fc + 1],
                    scale=1.0,
                )

            # dense: logitsT [53, 256] = sum_{j,fc} dwt[(j,fc)].T @ pooled_r
            lg_ps = cv_psum.tile([NREL, NS], f32, tag="cv", name="lgps")
            nmm = 0
            for j in range(3):
                for fc, (f0, fw) in enumerate(FCH):
                    nc.tensor.matmul(
                        out=lg_ps[:, :],
                        lhsT=dwt_sb[0:fw, (j * 2 + fc) * NREL:(j * 2 + fc + 1) * NREL],
                        rhs=pr[fc][0:fw, j, :],
                        start=(nmm == 0),
                        stop=(nmm == 5),
                    )
                    nmm += 1
            lg_sb = singles.tile([NREL, NS], bf16)
            nc.vector.tensor_copy(out=lg_sb[:, :], in_=lg_ps[:, :])

            # transpose logits -> [256 sents, 53]
            ls = [singles.tile([128, NREL], bf16, name=f"ls{c}") for c in range(2)]
            for sc in range(2):
                ltp = tp_psum.tile([128, 4, 128], bf16, tag="tp", name="ltp")
                nc.tensor.transpose(
                    out=ltp[0:128, 0, 0:NREL],
                    in_=lg_sb[:, sc * 128:(sc + 1) * 128],
                    identity=ident[0:NREL, 0:NREL],
                )
                nc.vector.tensor_copy(out=ls[sc][:, :], in_=ltp[0:128, 0, 0:NREL])

            # bag aggregation: bagT [128 bags, 53] per bag-chunk (+ dense bias/8)
            cc_dram = ctx.enter_context(tc.tile_pool(name="ccd", bufs=1, space="DRAM"))
            cc_in = cc_dram.tile([NBAGS, NREL], f32)
            cc_out = cc_dram.tile([NBAGS, NREL], f32)
            for bc in range(2):
                bg = cv_psum.tile([128, NREL], f32, tag="cv", name="bg")
                for sc in range(2):
                    nc.tensor.matmul(
                        out=bg[:, :],
                        lhsT=snorm_sb[sc][:, bc * 128:(bc + 1) * 128],
                        rhs=ls[sc][:, :],
                        start=(sc == 0),
                        stop=False,
                    )
                nc.tensor.matmul(
                    out=bg[:, :],
                    lhsT=ones_sb[0:1, 0:128],
                    rhs=dbias_sb[0:1, :],
                    start=False,
                    stop=True,
                )
                bg_sb = singles.tile([128, NREL], f32, name=f"bgs{bc}")
                nc.vector.tensor_copy(out=bg_sb[:, :], in_=bg[:, :])
                nc.sync.dma_start(out=cc_in[bc * 128:(bc + 1) * 128, :], in_=bg_sb[:, :])

            if debug:
                nc.sync.dma_start(out=dbg_bag[:, :], in_=cc_in[:, :])
            nc.gpsimd.collective_compute(
                "AllReduce",
                mybir.AluOpType.add,
                replica_groups=[list(range(NCORES))],
                ins=[cc_in.opt()],
                outs=[cc_out.opt()],
            )

            # softmax over the 53 relations
            for bc in range(2):
                t = singles.tile([128, NREL], f32, name=f"sm{bc}")
                nc.sync.dma_start(out=t[:, :], in_=cc_out[bc * 128:(bc + 1) * 128, :])
                nmax = singles.tile([128, 1], f32, name=f"nmax{bc}")
                nc.vector.reduce_max(out=nmax[:, :], in_=t[:, :], axis=AX.X, negate=True)
                ex = singles.tile([128, NREL], f32, name=f"ex{bc}")
                nc.scalar.activation(
                    out=ex[:, :], in_=t[:, :], func=AF.Exp, bias=nmax[:, :], scale=1.0
                )
                ssum = singles.tile([128, 1], f32, name=f"ssum{bc}")
                nc.vector.reduce_sum(out=ssum[:, :], in_=ex[:, :], axis=AX.X)
                rcp = singles.tile([128, 1], f32, name=f"rcp{bc}")
                nc.vector.reciprocal(out=rcp[:, :], in_=ssum[:, :])
                res = singles.tile([128, NREL], f32, name=f"res{bc}")
                nc.vector.tensor_scalar_mul(res[:, :], ex[:, :], rcp[:, :])
                nc.sync.dma_start(out=out_d[bc * 128:(bc + 1) * 128, :], in_=res[:, :])

    nc.compile()
    return nc


def _get_program():
    global _PROGRAM
    if _PROGRAM is None:
        _PROGRAM = _build_program()
    return _PROGRAM


def _pad_edge(a):
    return np.concatenate([a[:, :1], a, a[:, -1:]], axis=1)


def _token_layout(padded):
    """[NS, LP] int32 -> gather-index layout [128, NBLK*TILES_PER_BLK].

    Within each block: 8 subgroups of 4 sentences, each padded to 512 cols
    (pad index 0). idx[p, blk*32+i] = stream[blk][i*128+p]."""
    a = padded.reshape(NBLK, SG_PER_BLK, SGS * LP)
    tok = np.zeros((NBLK, SG_PER_BLK, SG_COLS), np.int32)
    tok[:, :, :SGS * LP] = a
    flat = tok.reshape(NBLK, TILES_PER_BLK, 128)
    return flat.transpose(2, 0, 1).reshape(128, NBLK * TILES_PER_BLK)


def kernel(**inputs):
    sentences = np.asarray(inputs["sentences"]).astype(np.int32)
    pos1 = np.asarray(inputs["pos1"]).astype(np.int32)
    pos2 = np.asarray(inputs["pos2"]).astype(np.int32)
    masks = np.asarray(inputs["masks"]).astype(np.float32)
    bag_ids = np.asarray(inputs["bag_ids"]).astype(np.int64)
    word_emb = np.asarray(inputs["word_emb"]).astype(np.float32)
    pf1_emb = np.asarray(inputs["pf1_emb"]).astype(np.float32)
    pf2_emb = np.asarray(inputs["pf2_emb"]).astype(np.float32)
    conv_w = np.asarray(inputs["conv_w"]).astype(np.float32)
    conv_b = np.asarray(inputs["conv_b"]).astype(np.float32)
    dense_w = np.asarray(inputs["dense_w"]).astype(np.float32)
    dense_b = np.asarray(inputs["dense_b"]).astype(np.float32)

    # ---- shared (replicated) parameter prep ----
    wemb_bf = word_emb.astype(BF16)

    w01 = (conv_w * 0.01).transpose(1, 0, 2)  # [310, 230, 3]
    wt = np.zeros((3, 128, 3 * NF), np.float32)
    for cc, (c0, cw) in enumerate(CCH):
        wt[cc, :cw, :] = w01[c0:c0 + cw].transpose(0, 2, 1).reshape(cw, 3 * NF)
    wt = wt.astype(BF16)

    dw100 = dense_w * 100.0  # [53, 690]
    dwt = np.zeros((128, 6 * NREL), np.float32)
    for j in range(3):
        for fc, (f0, fw) in enumerate(FCH):
            dwt[:fw, (j * 2 + fc) * NREL:(j * 2 + fc + 1) * NREL] = \
                dw100[:, j * NF + f0:j * NF + f0 + fw].T
    dwt = dwt.astype(BF16)

    actb = np.full((128, 2), -1.0, np.float32)
    for fc, (f0, fw) in enumerate(FCH):
        actb[:fw, fc] = 0.01 * conv_b[f0:f0 + fw] - 1.0

    dbias = (dense_b / NCORES).reshape(1, NREL).astype(BF16)

    counts = np.bincount(bag_ids, minlength=NBAGS).astype(np.float32)
    counts = np.maximum(counts, 1.0)

    # ---- per-core prep ----
    in_maps = []
    for r in range(NCORES):
        sl = slice(r * NS, (r + 1) * NS)
        idxw = _token_layout(_pad_edge(sentences[sl]))
        p1p = _pad_edge(pos1[sl])  # [NS, LP]
        p2p = _pad_edge(pos2[sl])
        pfv = np.concatenate([pf1_emb[p1p], pf2_emb[p2p]], axis=2)  # [NS, LP, 10]
        xpf = np.zeros((NBLK, SG_PER_BLK, SG_COLS, 2 * PD), np.float32)
        xpf[:, :, :SGS * LP, :] = pfv.reshape(NBLK, SG_PER_BLK, SGS * LP, 2 * PD)
        xpf = xpf.transpose(0, 3, 1, 2).reshape(NBLK, 2 * PD, BLK_COLS).astype(BF16)

        m = masks[sl]  # [256, 3, 120]
        md = np.stack([m[:, 0], m[:, 1] - m[:, 0], m[:, 2] - m[:, 1]], axis=1)
        masksd = md.reshape(NBLK, BLK, 3, L).transpose(0, 2, 1, 3) \
                   .reshape(3 * NBLK, BLK * L).astype(BF16)

        bags = bag_ids[sl]
        snorm = np.zeros((NS, NBAGS), np.float32)
        snorm[np.arange(NS), bags] = 1.0 / counts[bags]
        snorm = snorm.astype(BF16)

        in_maps.append({
            "wemb": wemb_bf,
            "idxw": idxw.astype(np.int32),
            "xpf": xpf,
            "masksd": masksd,
            "snorm": snorm,
            "wt": wt,
            "dwt": dwt,
            "actb": actb,
            "dbias": dbias,
        })

    nc = _get_program()
    from concourse.bass_utils import run_bass_kernel_spmd

    trace = bool(int(os.environ.get("KERNEL_TRACE", "0")))
    res = run_bass_kernel_spmd(
        nc, in_maps, core_ids=list(range(NCORES)), trace=trace
    )
    global LAST_RESULT
    LAST_RESULT = res
    return res.results[0]["out"].astype(np.float32)


if __name__ == "__main__":
    d = np.load("/root/problem/ref_inputs.npz")
    out = kernel(**{k: d[k] for k in d.files})
    print("out", out.shape, out.dtype)
    ref = np.load("/root/problem/ref_out.npy")
    err = np.abs(out - ref).max() / np.abs(ref).max()
    print("Relative error:", err)
